# revision 3
# baseline (speedup 1.0000x reference)
"""Trainium2 Bass kernel for the tiny-MLP ensemble problem.

Computes, for B collocation points x[B,3]:
    u[b, n, k] = tanh(sum_d x[b,d] * W[n,k,d] + b[n,k])     (N=64 layers, 3x3 each)
    out_j[b]   = sum_n c_j[n] * u[b, n, j]                  (j in {rho, p, u})

Strategy (pure data parallel over 8 cores, B/8 points each):
  - Host packs x (transposed + grouped, fp16) so the device sees a stacked
    layout: 32 point-groups x 4 rows (x0, x1, x2, 1.0) = 128 SBUF partitions;
    columns are points. The constant-1 row folds the bias into the matmul.
  - MM1 (bf16): block-diagonal stationary [128,128] computes 4 of the 192
    tanh-inputs for each of the 32 groups at once -> PSUM [128, 512] per
    slice (48 slices).
  - ScalarE tanh reads PSUM directly (batched slices per instruction),
    writes bf16 to SBUF.
  - MM2 (bf16): block-diagonal stationary [128,96] contracts the 4
    nk-channels per slice against the head vectors, accumulating all 48
    slices into one PSUM [96, 512] tile (rows = 3 heads x 32 groups).
  - VectorE copies the accumulated heads to SBUF; DMA out; host un-permutes.
"""

import numpy as np

import concourse.bacc as bacc
import concourse.bass as bass
import concourse.mybir as mybir
import concourse.tile as tile
from concourse.bass_utils import run_bass_kernel_spmd

F32 = mybir.dt.float32
F16 = mybir.dt.float16
NP_F16 = np.float16

N_CORES = 8
B_FULL = 1_000_000
NL, D = 64, 3
NK = NL * D            # 192 tanh channels per point
G = 32                 # point groups stacked on partitions
CW = 512               # points per group per supergroup
KOUT = 4               # nk channels per group per slice
NSLICES = NK // KOUT   # 48
SPA = 3                # slices batched per activation instruction (3 PSUM banks)
PTS_PER_SG = G * CW    # 16384
PER_CORE_RAW = B_FULL // N_CORES      # 125000
SG = -(-PER_CORE_RAW // PTS_PER_SG)   # 8 supergroups
PER_CORE = SG * PTS_PER_SG            # 131072 (padded)
COLS = SG * CW                        # 4096 device columns per core

_NC_CACHE = {}

PROFILE = False
LAST_RESULT = None


def build_nc(sg=SG):
    cols = sg * CW
    nc = bacc.Bacc("TRN2", target_bir_lowering=False, debug=False,
                   num_devices=N_CORES)
    x4 = nc.dram_tensor("x4", [4 * G, cols], F16, kind="ExternalInput")
    w1 = nc.dram_tensor("w1", [4 * G, NSLICES * 128], F16, kind="ExternalInput")
    w2 = nc.dram_tensor("w2", [128, NSLICES * 3 * G], F16, kind="ExternalInput")
    out = nc.dram_tensor("out", [3 * G, cols], F32, kind="ExternalOutput")

    with tile.TileContext(nc) as tc:
        with (
            tc.tile_pool(name="wpool", bufs=1) as wpool,
            tc.tile_pool(name="xpool", bufs=3) as xpool,
            tc.tile_pool(name="upool", bufs=3) as upool,
            tc.tile_pool(name="opool", bufs=2) as opool,
            tc.tile_pool(name="pu", bufs=2, space=bass.MemorySpace.PSUM) as pupool,
            tc.tile_pool(name="po", bufs=2, space=bass.MemorySpace.PSUM) as popool,
        ):
            w1_sb = wpool.tile([4 * G, NSLICES * 128], F16, tag="w1")
            w2_sb = wpool.tile([128, NSLICES * 3 * G], F16, tag="w2")
            nc.sync.dma_start(out=w1_sb[:], in_=w1[:])
            nc.sync.dma_start(out=w2_sb[:], in_=w2[:])

            for t in range(sg):
                xt = xpool.tile([4 * G, CW], F16)
                nc.sync.dma_start(out=xt[:], in_=x4[:, t * CW:(t + 1) * CW])
                po = popool.tile([3 * G, CW], F32)
                for m in range(NSLICES // SPA):
                    pu = pupool.tile([128, SPA * CW], F32)
                    for q in range(SPA):
                        s = m * SPA + q
                        nc.tensor.matmul(
                            pu[:, q * CW:(q + 1) * CW],
                            w1_sb[:, s * 128:(s + 1) * 128],
                            xt[:],
                            start=True, stop=True,
                        )
                    ut = upool.tile([128, SPA * CW], F16)
                    nc.scalar.activation(
                        ut[:], pu[:], mybir.ActivationFunctionType.Tanh)
                    for q in range(SPA):
                        s = m * SPA + q
                        nc.tensor.matmul(
                            po[:],
                            w2_sb[:, s * 3 * G:(s + 1) * 3 * G],
                            ut[:, q * CW:(q + 1) * CW],
                            start=(s == 0), stop=(s == NSLICES - 1),
                            skip_group_check=True,
                        )
                ot = opool.tile([3 * G, CW], F32)
                nc.vector.tensor_copy(ot[:], po[:])
                nc.sync.dma_start(out=out[:, t * CW:(t + 1) * CW], in_=ot[:])

    nc.compile()
    return nc


def get_nc(sg=SG):
    if sg not in _NC_CACHE:
        _NC_CACHE[sg] = build_nc(sg)
    return _NC_CACHE[sg]


def pack_weights(W, b, c_rho, c_p, c_u):
    """Build the two block-diagonal stationary matrices (all slices packed)."""
    W = np.asarray(W, np.float32)
    b = np.asarray(b, np.float32)
    # Wcat[d, 3n+k] = W[n, k, d]
    wcat = np.ascontiguousarray(W.transpose(2, 0, 1)).reshape(3, NK)
    bflat = np.asarray(b, np.float32).reshape(NK)
    # C[3n+j, j] = c_j[n]
    C = np.zeros((NK, 3), np.float32)
    C[np.arange(NL) * 3 + 0, 0] = np.asarray(c_rho, np.float32).reshape(NL)
    C[np.arange(NL) * 3 + 1, 1] = np.asarray(c_p, np.float32).reshape(NL)
    C[np.arange(NL) * 3 + 2, 2] = np.asarray(c_u, np.float32).reshape(NL)

    gi = np.arange(G)
    # w1[(4g+d), s*128 + 4g+o] = Wcat[d, 4s+o]  (d<3);  = bflat[4s+o] (d==3)
    w1 = np.zeros((G, 4, NSLICES, G, KOUT), np.float32)
    w1[gi, :3, :, gi, :] = wcat.reshape(3, NSLICES, KOUT)[None]
    w1[gi, 3, :, gi, :] = bflat.reshape(NSLICES, KOUT)[None]
    w1 = np.ascontiguousarray(w1).reshape(4 * G, NSLICES * 128)

    # w2[(4g+o), s*96 + 3g+j] = C[4s+o, j]
    w2 = np.zeros((G, KOUT, NSLICES, G, 3), np.float32)
    w2[gi, :, :, gi, :] = C.reshape(NSLICES, KOUT, 3).transpose(1, 0, 2)[None]
    w2 = np.ascontiguousarray(w2).reshape(128, NSLICES * 3 * G)
    return w1.astype(NP_F16), w2.astype(NP_F16)


def pack_x_core(x_core_padded, sg=SG):
    """[sg*16384, 3] -> [128, sg*512] stacked layout with constant-1 rows."""
    xc = x_core_padded.reshape(sg, G, CW, 3).transpose(1, 3, 0, 2)  # [G,3,sg,CW]
    x4 = np.ones((G, 4, sg, CW), np.float32)
    x4[:, :3] = xc
    return np.ascontiguousarray(x4).reshape(4 * G, sg * CW).astype(NP_F16)


def unpack_out_core(out_dev, sg=SG):
    """[96, sg*512] -> [sg*16384, 3]."""
    o = out_dev.reshape(G, 3, sg, CW).transpose(2, 0, 3, 1)  # [sg,G,CW,3]
    return np.ascontiguousarray(o).reshape(sg * PTS_PER_SG, 3)


def kernel(x, W, b, c_rho, c_p, c_u):
    x = np.asarray(x, np.float32)
    nc = get_nc()
    w1, w2 = pack_weights(W, b, c_rho, c_p, c_u)

    in_maps = []
    for c in range(N_CORES):
        off = c * PER_CORE_RAW
        xc = np.zeros((PER_CORE, 3), np.float32)
        xc[:PER_CORE_RAW] = x[off:off + PER_CORE_RAW]
        in_maps.append({"x4": pack_x_core(xc), "w1": w1, "w2": w2})

    res = run_bass_kernel_spmd(nc, in_maps, list(range(N_CORES)),
                               trace=PROFILE)
    if PROFILE:
        globals()["LAST_RESULT"] = res
    outs = []
    for c in range(N_CORES):
        outs.append(unpack_out_core(res.results[c]["out"])[:PER_CORE_RAW])
    full = np.concatenate(outs, axis=0)  # [1M, 3]
    return (np.ascontiguousarray(full[:, 0:1]),
            np.ascontiguousarray(full[:, 1:2]),
            np.ascontiguousarray(full[:, 2:3]))


# revision 11
# speedup vs baseline: 1.0689x; 1.0689x over previous
"""Trainium2 Bass kernel for the tiny-MLP ensemble problem.

Computes, for B collocation points x[B,3]:
    u[b, n, k] = tanh(sum_d x[b,d] * W[n,k,d] + b[n,k])     (N=64 layers, 3x3 each)
    out_j[b]   = sum_n c_j[n] * u[b, n, j]                  (j in {rho, p, u})

Strategy (pure data parallel over 8 cores, B/8 points each):
  - Host packs x (transposed + grouped, fp16) so the device sees a stacked
    layout: 32 point-groups x 4 rows (x0, x1, x2, 1.0) = 128 SBUF partitions;
    columns are points. The constant-1 row folds the bias into the matmul.
  - MM1 (bf16): block-diagonal stationary [128,128] computes 4 of the 192
    tanh-inputs for each of the 32 groups at once -> PSUM [128, 512] per
    slice (48 slices).
  - ScalarE tanh reads PSUM directly (batched slices per instruction),
    writes bf16 to SBUF.
  - MM2 (bf16): block-diagonal stationary [128,96] contracts the 4
    nk-channels per slice against the head vectors, accumulating all 48
    slices into one PSUM [96, 512] tile (rows = 3 heads x 32 groups).
  - VectorE copies the accumulated heads to SBUF; DMA out; host un-permutes.
"""

import numpy as np

import concourse.bacc as bacc
import concourse.bass as bass
import concourse.mybir as mybir
import concourse.tile as tile
from concourse.bass_utils import run_bass_kernel_spmd

F32 = mybir.dt.float32
F16 = mybir.dt.float16
NP_F16 = np.float16

N_CORES = 8
B_FULL = 1_000_000
NL, D = 64, 3
NK = NL * D            # 192 tanh channels per point
G = 32                 # point groups stacked on partitions
CW = 512               # points per group per supergroup
KOUT = 4               # nk channels per group per slice
NSLICES = NK // KOUT   # 48
SPA = 3                # slices batched per activation instruction (3 PSUM banks)
PER_CORE_RAW = B_FULL // N_CORES      # 125000
# Per-supergroup column widths: 7 full 512-wide SGs plus a 323-wide tail so
# the padded per-core point count is 32*3907 = 125024 (0.02% waste).
WIDTHS = [CW] * 7 + [323]
COLS = sum(WIDTHS)                    # 3907 device columns per core
SG = len(WIDTHS)
PER_CORE = G * COLS                   # 125024 (padded)
COL_OFF = [sum(WIDTHS[:i]) for i in range(SG)]

_NC_CACHE = {}

PROFILE = False
LAST_RESULT = None


def build_nc():
    nc = bacc.Bacc("TRN2", target_bir_lowering=False, debug=False,
                   num_devices=N_CORES)
    x4 = nc.dram_tensor("x4", [4 * G, COLS], F16, kind="ExternalInput")
    w1 = nc.dram_tensor("w1", [4 * G, NSLICES * 128], F16, kind="ExternalInput")
    w2 = nc.dram_tensor("w2", [128, NSLICES * 3 * G], F16, kind="ExternalInput")
    out = nc.dram_tensor("out", [3 * G, COLS], F16, kind="ExternalOutput")

    with tile.TileContext(nc) as tc:
        with (
            tc.tile_pool(name="wpool", bufs=1) as wpool,
            tc.tile_pool(name="xpool", bufs=3) as xpool,
            tc.tile_pool(name="upool", bufs=3) as upool,
            tc.tile_pool(name="opool", bufs=2) as opool,
            tc.tile_pool(name="pu", bufs=2, space=bass.MemorySpace.PSUM) as pupool,
            tc.tile_pool(name="po", bufs=2, space=bass.MemorySpace.PSUM) as popool,
        ):
            w1_sb = wpool.tile([4 * G, NSLICES * 128], F16, tag="w1")
            w2_sb = wpool.tile([128, NSLICES * 3 * G], F16, tag="w2")

            # First x chunk goes out on the sync queue ahead of everything;
            # weights stream in per-m-group chunks on the otherwise-idle
            # vector/gpsimd DMA queues so the first matmul can start as soon
            # as x chunk 0 plus weight chunk 0 have landed.
            xts = {}
            xts[0] = xpool.tile([4 * G, WIDTHS[0]], F16, name="xt0")
            nc.sync.dma_start(out=xts[0][:], in_=x4[:, 0:WIDTHS[0]])
            NM = NSLICES // SPA
            WCH = 8  # weight DMA chunks
            for ch in range(WCH):
                c0 = ch * (NSLICES // WCH) * 128
                c1 = (ch + 1) * (NSLICES // WCH) * 128
                nc.gpsimd.dma_start(out=w1_sb[:, c0:c1], in_=w1[:, c0:c1])
            for ch in range(WCH):
                c0 = ch * (NSLICES // WCH) * 3 * G
                c1 = (ch + 1) * (NSLICES // WCH) * 3 * G
                nc.sync.dma_start(out=w2_sb[:, c0:c1], in_=w2[:, c0:c1])

            for t in range(SG):
                w = WIDTHS[t]
                o0 = COL_OFF[t]
                if t not in xts:
                    xts[t] = xpool.tile([4 * G, w], F16, name=f"xt{t}")
                    nc.sync.dma_start(out=xts[t][:], in_=x4[:, o0:o0 + w])
                xt = xts[t]
                po = popool.tile([3 * G, w], F32)
                for m in range(NM):
                    # [128, SPA, CW] keeps each matmul's output slice aligned
                    # to a PSUM bank even when w < CW (matmul output must not
                    # cross a bank boundary).
                    pu = pupool.tile([128, SPA, CW], F32)
                    for q in range(SPA):
                        s = m * SPA + q
                        nc.tensor.matmul(
                            pu[:, q, 0:w],
                            w1_sb[:, s * 128:(s + 1) * 128],
                            xt[:],
                            start=True, stop=True,
                        )
                    ut = upool.tile([128, SPA, w], F16)
                    nc.scalar.activation(
                        ut[:], pu[:, :, 0:w],
                        mybir.ActivationFunctionType.Tanh)
                    for q in range(SPA):
                        s = m * SPA + q
                        nc.tensor.matmul(
                            po[:],
                            w2_sb[:, s * 3 * G:(s + 1) * 3 * G],
                            ut[:, q, :],
                            start=(s == 0), stop=(s == NSLICES - 1),
                            skip_group_check=True,
                        )
                ot = opool.tile([3 * G, w], F16)
                nc.vector.tensor_copy(ot[:], po[:])
                nc.sync.dma_start(out=out[:, o0:o0 + w], in_=ot[:])

    nc.compile()
    return nc


def get_nc():
    if 0 not in _NC_CACHE:
        _NC_CACHE[0] = build_nc()
    return _NC_CACHE[0]


def pack_weights(W, b, c_rho, c_p, c_u):
    """Build the two block-diagonal stationary matrices (all slices packed)."""
    W = np.asarray(W, np.float32)
    b = np.asarray(b, np.float32)
    # Wcat[d, 3n+k] = W[n, k, d]
    wcat = np.ascontiguousarray(W.transpose(2, 0, 1)).reshape(3, NK)
    bflat = np.asarray(b, np.float32).reshape(NK)
    # C[3n+j, j] = c_j[n]
    C = np.zeros((NK, 3), np.float32)
    C[np.arange(NL) * 3 + 0, 0] = np.asarray(c_rho, np.float32).reshape(NL)
    C[np.arange(NL) * 3 + 1, 1] = np.asarray(c_p, np.float32).reshape(NL)
    C[np.arange(NL) * 3 + 2, 2] = np.asarray(c_u, np.float32).reshape(NL)

    gi = np.arange(G)
    # w1[(4g+d), s*128 + 4g+o] = Wcat[d, 4s+o]  (d<3);  = bflat[4s+o] (d==3)
    w1 = np.zeros((G, 4, NSLICES, G, KOUT), np.float32)
    w1[gi, :3, :, gi, :] = wcat.reshape(3, NSLICES, KOUT)[None]
    w1[gi, 3, :, gi, :] = bflat.reshape(NSLICES, KOUT)[None]
    w1 = np.ascontiguousarray(w1).reshape(4 * G, NSLICES * 128)

    # w2[(4g+o), s*96 + 3g+j] = C[4s+o, j]
    w2 = np.zeros((G, KOUT, NSLICES, G, 3), np.float32)
    w2[gi, :, :, gi, :] = C.reshape(NSLICES, KOUT, 3).transpose(1, 0, 2)[None]
    w2 = np.ascontiguousarray(w2).reshape(128, NSLICES * 3 * G)
    return w1.astype(NP_F16), w2.astype(NP_F16)


def pack_x_core(x_core_padded):
    """[PER_CORE, 3] -> [128, COLS] stacked layout with constant-1 rows."""
    parts = []
    off = 0
    for w in WIDTHS:
        blk = x_core_padded[off:off + G * w].reshape(G, w, 3)
        x4 = np.ones((G, 4, w), np.float32)
        x4[:, :3] = blk.transpose(0, 2, 1)
        parts.append(x4.reshape(4 * G, w))
        off += G * w
    return np.ascontiguousarray(np.concatenate(parts, 1)).astype(NP_F16)


def unpack_out_core(out_dev):
    """[96, COLS] (f16) -> [PER_CORE, 3] (f32)."""
    parts = []
    for t, w in enumerate(WIDTHS):
        o0 = COL_OFF[t]
        o = out_dev[:, o0:o0 + w].astype(np.float32)
        parts.append(o.reshape(G, 3, w).transpose(0, 2, 1).reshape(G * w, 3))
    return np.concatenate(parts, 0)


def kernel(x, W, b, c_rho, c_p, c_u):
    x = np.asarray(x, np.float32)
    nc = get_nc()
    w1, w2 = pack_weights(W, b, c_rho, c_p, c_u)

    in_maps = []
    for c in range(N_CORES):
        off = c * PER_CORE_RAW
        xc = np.zeros((PER_CORE, 3), np.float32)
        xc[:PER_CORE_RAW] = x[off:off + PER_CORE_RAW]
        in_maps.append({"x4": pack_x_core(xc), "w1": w1, "w2": w2})

    res = run_bass_kernel_spmd(nc, in_maps, list(range(N_CORES)),
                               trace=PROFILE)
    if PROFILE:
        globals()["LAST_RESULT"] = res
    outs = []
    for c in range(N_CORES):
        outs.append(unpack_out_core(res.results[c]["out"])[:PER_CORE_RAW])
    full = np.concatenate(outs, axis=0)  # [1M, 3]
    return (np.ascontiguousarray(full[:, 0:1]),
            np.ascontiguousarray(full[:, 1:2]),
            np.ascontiguousarray(full[:, 2:3]))
